# revision 15
# baseline (speedup 1.0000x reference)
"""AutoDiscretizationEmbedding kernel for 8 Trainium2 NeuronCores.

Math per token t (x_t scalar):  h = leaky_relu(x_t*w1 + b1, 0.1);
logits = h + h@w2.T + b2;  out_t = softmax(logits) @ emb.

Engine mapping (per 512-token chunk, bins-on-partitions "^T" layout):
  GpSimd: xb = broadcast x row to 100 partitions;
          h  = xb*w1 + b1 (tensor_scalar, two per-partition scalars)
          -- the rank-1 "mm1" runs on the otherwise-idle GpSimd engine so the
          PE never stalls waiting for hT.
  DVE:    hT = max(h, 0.1h)  (one scalar_tensor_tensor; exact leaky --
          the ACT Lrelu table costs ~1e-2 absmax error, do not use it)
  PE:     l_ps = W2'.T @ hT          (fp32r, W2' = w2 + I folds the residual)
  ACT:    uT = exp(l_ps + b2)        (per-partition bias; sole table set)
  PE:     per 128-token subtile j:
            o_ps[128,512] = uT_j.T @ emb     (fp32r, 1 cyc/row)
            z_ps[128,4]   = uT_j.T @ ones    (softmax normalizer, hidden
                                              under the o matmul)
  DVE:    rc = 1/z ;  normalize+evict o_ps -> ost staging
  ACT:    (half the normalizes, Copy with per-partition scale = rc)
  DMA:    1 MiB store per chunk, staged in SBUF.
Host folds: W2' pre-transposed into lhsT layout, w1/b1 as per-partition
columns, ones columns appended to emb. Softmax needs no max-shift (logits
bounded ~[-3,3]).  Data-parallel over the 65536 tokens, 8192 per core.
"""

import numpy as np

B, S = 8, 8192
BINS, DIM = 100, 512
NCORES = 8
NTOK = (B * S) // NCORES
CHUNK = 512
NSUB = CHUNK // 128
NCH = NTOK // CHUNK

_CACHE = {}


def _build_nc():
    import concourse.tile as tile
    from concourse import bacc, mybir

    f32 = mybir.dt.float32
    fr = mybir.dt.float32r
    Act = mybir.ActivationFunctionType
    Alu = mybir.AluOpType

    nc = bacc.Bacc("TRN2", target_bir_lowering=False, debug=False,
                   num_devices=NCORES)
    xo_d = nc.dram_tensor("xo", [1, NTOK], f32, kind="ExternalInput").ap()
    w1b1_d = nc.dram_tensor("w1b1", [BINS, 2], f32, kind="ExternalInput").ap()
    w2ti_d = nc.dram_tensor("w2ti", [BINS, BINS], fr, kind="ExternalInput").ap()
    b2c_d = nc.dram_tensor("b2c", [BINS, 1], f32, kind="ExternalInput").ap()
    embo_d = nc.dram_tensor("embo", [BINS, DIM + 4], fr, kind="ExternalInput").ap()
    out_d = nc.dram_tensor("out", [NTOK, DIM], f32, kind="ExternalOutput").ap()

    with tile.TileContext(nc) as tc:
        with (
            tc.tile_pool(name="const", bufs=1) as cpool,
            tc.tile_pool(name="xb", bufs=2) as xbpool,
            tc.tile_pool(name="hsb", bufs=2) as hspool,
            tc.tile_pool(name="hT", bufs=4) as hpool,
            tc.tile_pool(name="uT", bufs=3) as upool,
            tc.tile_pool(name="ost", bufs=4) as opool,
            tc.tile_pool(name="rc", bufs=8) as rpool,
            tc.tile_pool(name="pl", bufs=2, space="PSUM") as pl,
            tc.tile_pool(name="po", bufs=5, space="PSUM") as po,
            tc.tile_pool(name="pz", bufs=1, space="PSUM") as pz,
        ):
            xo = cpool.tile([1, NTOK], f32)
            nc.sync.dma_start(xo[:], xo_d[:])
            w1b1 = cpool.tile([BINS, 2], f32)
            nc.sync.dma_start(w1b1[:], w1b1_d[:])
            w2ti = cpool.tile([BINS, BINS], fr)
            nc.sync.dma_start(w2ti[:], w2ti_d[:])
            b2c = cpool.tile([BINS, 1], f32)
            nc.sync.dma_start(b2c[:], b2c_d[:])
            embo = cpool.tile([BINS, DIM + 4], fr)
            nc.sync.dma_start(embo[:], embo_d[:])

            for ch in range(NCH):
                t0 = ch * CHUNK
                xb = xbpool.tile([BINS, CHUNK], f32)
                nc.gpsimd.partition_broadcast(xb[:], xo[0:1, t0:t0 + CHUNK],
                                              channels=BINS)
                h_sb = hspool.tile([BINS, CHUNK], f32)
                nc.gpsimd.tensor_scalar(h_sb[:], xb[:], w1b1[:, 0:1],
                                        w1b1[:, 1:2], op0=Alu.mult, op1=Alu.add)
                hT = hpool.tile([BINS, CHUNK], fr)
                nc.vector.scalar_tensor_tensor(hT[:], h_sb[:], 0.1, h_sb[:],
                                               op0=Alu.mult, op1=Alu.max)

                l_ps = pl.tile([BINS, CHUNK], f32)
                nc.tensor.matmul(l_ps[:], w2ti[:], hT[:], start=True, stop=True)
                uT = upool.tile([BINS, CHUNK], fr)
                nc.scalar.activation(uT[:], l_ps[:], Act.Exp, bias=b2c[:])

                ost = opool.tile([128, NSUB * DIM], f32)
                for j in range(NSUB):
                    u_j = uT[:, j * 128:(j + 1) * 128]
                    o_ps = po.tile([128, DIM], f32)
                    nc.tensor.matmul(o_ps[:], u_j, embo[:, 0:DIM],
                                     start=True, stop=True)
                    z_ps = pz.tile([128, 4], f32)
                    nc.tensor.matmul(z_ps[:], u_j, embo[:, DIM:DIM + 4],
                                     start=True, stop=True)
                    rc = rpool.tile([128, 1], f32)
                    nc.vector.reciprocal(rc[:], z_ps[:, 0:1])
                    dst = ost[:, j * DIM:(j + 1) * DIM]
                    if j % 2 == 0:
                        nc.scalar.activation(dst, o_ps[:], Act.Copy, scale=rc[:])
                    else:
                        nc.vector.tensor_scalar_mul(dst, o_ps[:], rc[:])

                out_view = out_d[t0:t0 + CHUNK, :].rearrange(
                    "(a p) d -> p a d", p=128)
                nc.sync.dma_start(
                    out_view, ost[:].rearrange("p (a d) -> p a d", d=DIM))
    nc.compile()
    return nc


def _prep_in_maps(x, w1, b1, w2, b2, emb):
    x = np.ascontiguousarray(np.asarray(x, dtype=np.float32)).reshape(B * S)
    w1 = np.asarray(w1, dtype=np.float32)
    b1 = np.asarray(b1, dtype=np.float32)
    w2 = np.asarray(w2, dtype=np.float32)
    b2 = np.asarray(b2, dtype=np.float32)
    emb = np.asarray(emb, dtype=np.float32)

    w1b1 = np.ascontiguousarray(np.stack([w1[:, 0], b1], axis=1))    # [BINS, 2]
    w2ti = np.ascontiguousarray((w2 + np.eye(BINS, dtype=np.float32)).T)
    b2c = np.ascontiguousarray(b2.reshape(BINS, 1))
    embo = np.ascontiguousarray(
        np.concatenate([emb, np.ones((BINS, 4), np.float32)], axis=1))

    in_maps = []
    for c in range(NCORES):
        xo = np.ascontiguousarray(x[c * NTOK:(c + 1) * NTOK][None, :])
        in_maps.append({"xo": xo, "w1b1": w1b1, "w2ti": w2ti,
                        "b2c": b2c, "embo": embo})
    return in_maps


def _run(in_maps, trace=False, **kw):
    from concourse.bass_utils import run_bass_kernel_spmd
    if "nc" not in _CACHE:
        _CACHE["nc"] = _build_nc()
    return run_bass_kernel_spmd(_CACHE["nc"], in_maps,
                                list(range(NCORES)), trace=trace, **kw)


def kernel(**inputs):
    in_maps = _prep_in_maps(inputs["x"], inputs["w1"], inputs["b1"],
                            inputs["w2"], inputs["b2"], inputs["emb"])
    res = _run(in_maps)
    out = np.stack([res.results[c]["out"] for c in range(NCORES)])
    return out.reshape(B, S, DIM).astype(np.float32, copy=False)
